# revision 1
# baseline (speedup 1.0000x reference)
"""PatchSelfAttentionBlock kernel for Trainium2 (8 NeuronCores, data-parallel over batch).

Per core (one batch element): x[512,1024] +2D-sinusoidal PE, QKV projections,
8-head softmax attention over 1024 tokens, output projection + bias.

v2 strategy (bf16 matmuls, fp32 PSUM):
  - Scores computed transposed (S^T = K^T Q) so exp feeds PV directly.
  - The two heads of a 128-channel tile run their score matmuls in PE row
    tiles (0,0)/(64,0) back-to-back -> the 64-contraction matmuls stream
    concurrently (~2x measured on the PE array).
  - V^T carries an appended ones column per head (PSUM row 64 = softmax
    denominator, free with the PV matmul).
  - Scalar engine does EXP only; all DMA issues live on sync/vector/gpsimd.
  - The pair pipeline keeps scalar fed: per mt slot the PE does the score
    pair + one PV chunk of the previous pair, plus one QKV-projection
    filler unit early on.
  - Attention output is stacked UNNORMALIZED in bf16; normalization (a
    per-query-column scale) is applied afterwards via a DMA-broadcast
    reciprocal (last pair: PE E-matrix broadcast to skip the DRAM bounce).
  - Output projection + bias written back as bf16, cast to f32 on host.
"""

import math
import sys

sys.path.insert(0, "/opt/trn_rl_repo")

import numpy as np
import ml_dtypes

import concourse.bass as bass
import concourse.mybir as mybir
import concourse.tile as tile
from concourse import bacc
from concourse.bass_utils import run_bass_kernel_spmd

B, C, H, W = 8, 512, 32, 32
N = H * W          # 1024 tokens
NH = 8             # heads
D = 64             # head dim
CT = C // 128      # 4 channel tiles
MT = N // 128      # 8 token tiles (keys)
NC_ = 512          # query chunk size (one PSUM bank of fp32)
NP = NH // 2       # 4 head pairs (one per channel tile)
SCALE = 1.0 / math.sqrt(D)

F32 = mybir.dt.float32
BF16 = mybir.dt.bfloat16


def _pe_table():
    half = C // 2
    div = np.exp(np.arange(0, half, 2, dtype=np.float32) * (-math.log(10000.0) / half))
    pos_h = np.arange(H, dtype=np.float32)[:, None] * div[None, :]
    pos_w = np.arange(W, dtype=np.float32)[:, None] * div[None, :]
    emb_h = np.zeros((half, H), dtype=np.float32)
    emb_h[0::2] = np.sin(pos_h).T
    emb_h[1::2] = np.cos(pos_h).T
    emb_w = np.zeros((half, W), dtype=np.float32)
    emb_w[0::2] = np.sin(pos_w).T
    emb_w[1::2] = np.cos(pos_w).T
    pe = np.concatenate(
        [
            np.broadcast_to(emb_h[:, :, None], (half, H, W)),
            np.broadcast_to(emb_w[:, None, :], (half, H, W)),
        ],
        axis=0,
    )
    return np.ascontiguousarray(pe.reshape(C, N))


def _build_program():
    nc = bacc.Bacc("TRN2", target_bir_lowering=False, debug=False, num_devices=B)

    x_ext = nc.dram_tensor("x", [C, N], BF16, kind="ExternalInput").ap()
    em_ext = nc.dram_tensor("ematrix", [2, 128], BF16, kind="ExternalInput").ap()
    wqT_ext = nc.dram_tensor("wqT", [C, C], BF16, kind="ExternalInput").ap()
    wkT_ext = nc.dram_tensor("wkT", [C, C], BF16, kind="ExternalInput").ap()
    wvT_ext = nc.dram_tensor("wvT", [C, C], BF16, kind="ExternalInput").ap()
    woT_ext = nc.dram_tensor("woT", [C, C], BF16, kind="ExternalInput").ap()
    bo_ext = nc.dram_tensor("bo", [C, 1], F32, kind="ExternalInput").ap()
    y_ext = nc.dram_tensor("y", [C, N], BF16, kind="ExternalOutput").ap()

    with tile.TileContext(nc) as tc:
        with (
            tc.tile_pool(name="consts", bufs=1) as consts,
            tc.tile_pool(name="xin", bufs=1) as xin_pool,
            tc.tile_pool(name="xpe", bufs=1) as xpe_pool,
            tc.tile_pool(name="qk", bufs=1) as qk_pool,
            tc.tile_pool(name="vt", bufs=1) as vt_pool,
            tc.tile_pool(name="e", bufs=34) as e_pool,
            tc.tile_pool(name="attn", bufs=1) as attn_pool,
            tc.tile_pool(name="norm", bufs=1) as norm_pool,
            tc.tile_pool(name="tb", bufs=2) as tb_pool,
            tc.tile_pool(name="ysb", bufs=4) as y_pool,
            tc.tile_pool(name="dram", bufs=1, space="DRAM") as dram_pool,
            tc.tile_pool(name="st_ps", bufs=3, space="PSUM") as st_ps,
            tc.tile_pool(name="pv_ps", bufs=1, space="PSUM") as pv_ps,
        ):
            # ---- input loads, ordered by first use. x already carries the
            # positional encoding (host-side add). 64KB chunks engage many
            # DMA engines; wq0/wk0 interleave on sync right after x0 ----
            x_t = []
            for ct in range(CT):
                xt = xin_pool.tile([128, N], BF16, tag=f"x{ct}", name=f"x{ct}")
                x_t.append(xt)
            wq_t, wk_t, wv_t, wo_t = [], [], [], []
            for ct in range(CT):
                t = consts.tile([128, C], BF16, tag=f"wq{ct}", name=f"wq{ct}")
                wq_t.append(t)
                t = consts.tile([128, C], BF16, tag=f"wk{ct}", name=f"wk{ct}")
                wk_t.append(t)

            def _x_chunk(eng, ct, q):
                sl = slice(256 * q, 256 * (q + 1))
                eng.dma_start(x_t[ct][:, sl], x_ext[128 * ct : 128 * (ct + 1), sl])

            def _w_chunk(eng, wt, wext, h):
                eng.dma_start(
                    wt[:, 256 * h : 256 * (h + 1)], wext[0:128, 256 * h : 256 * (h + 1)]
                )

            _x_chunk(nc.sync, 0, 0); _x_chunk(nc.sync, 0, 1)
            _x_chunk(nc.scalar, 0, 2); _x_chunk(nc.scalar, 0, 3)
            _w_chunk(nc.sync, wq_t[0], wqT_ext, 0)
            _w_chunk(nc.scalar, wq_t[0], wqT_ext, 1)
            _w_chunk(nc.sync, wk_t[0], wkT_ext, 0)
            _w_chunk(nc.scalar, wk_t[0], wkT_ext, 1)
            for ct in range(1, CT):
                _x_chunk(nc.sync, ct, 0); _x_chunk(nc.sync, ct, 1)
                _x_chunk(nc.scalar, ct, 2); _x_chunk(nc.scalar, ct, 3)
            vt_bf = [
                vt_pool.tile([128, NH * (D + 1)], BF16, tag=f"vt{m}", name=f"vt{m}")
                for m in range(MT)
            ]
            for m in range(MT):
                nc.gpsimd.memset(vt_bf[m][:], 1.0)
            for ct in range(1, CT):
                nc.gpsimd.dma_start(wq_t[ct][:], wqT_ext[128 * ct : 128 * (ct + 1), :])
                nc.gpsimd.dma_start(wk_t[ct][:], wkT_ext[128 * ct : 128 * (ct + 1), :])
            for ct in range(CT):
                t = consts.tile([128, C], BF16, tag=f"wv{ct}", name=f"wv{ct}")
                nc.gpsimd.dma_start(t[:], wvT_ext[128 * ct : 128 * (ct + 1), :])
                wv_t.append(t)
            em_sb = consts.tile([2, 128], BF16, tag="em")
            nc.gpsimd.dma_start(em_sb[:], em_ext[:])
            for ct in range(CT):
                t = consts.tile([128, C], BF16, tag=f"wo{ct}", name=f"wo{ct}")
                nc.gpsimd.dma_start(t[:], woT_ext[128 * ct : 128 * (ct + 1), :])
                wo_t.append(t)
            bo_t = []
            for ct in range(CT):
                t = consts.tile([128, 1], F32, tag=f"bo{ct}", name=f"bo{ct}")
                nc.gpsimd.dma_start(t[:], bo_ext[128 * ct : 128 * (ct + 1), :])
                bo_t.append(t)

            warm = norm_pool.tile([1, 8], F32, tag="warm")
            nc.gpsimd.memset(warm[:], 0.0)
            warm2 = norm_pool.tile([1, 8], BF16, tag="warm2")
            nc.scalar.activation(
                warm2[:], warm[:], mybir.ActivationFunctionType.Exp, scale=SCALE
            )

            xpe_bf = x_t

            q_bf = [qk_pool.tile([128, N], BF16, tag=f"q{c}", name=f"q{c}") for c in range(CT)]
            k_bf = [qk_pool.tile([128, N], BF16, tag=f"k{c}", name=f"k{c}") for c in range(CT)]
            nonce = [0]

            def _ps_tile(pool, tag):
                nonce[0] += 1
                return pool.tile([128, N], F32, tag=tag, name=f"ps{nonce[0]}")

            def proj_qk(w_t, dst, ct, pool, tag):
                """dst[ct] = (w.T @ xpe) for one 128-row output tile."""
                ps = _ps_tile(pool, tag)
                for kc in range(CT):
                    for nch in range(2):
                        nc.tensor.matmul(
                            ps[:, NC_ * nch : NC_ * (nch + 1)],
                            w_t[kc][:, 128 * ct : 128 * (ct + 1)],
                            xpe_bf[kc][:, NC_ * nch : NC_ * (nch + 1)],
                            start=(kc == 0),
                            stop=(kc == CT - 1),
                        )
                for nch in range(2):
                    nc.vector.tensor_copy(
                        dst[ct][:, NC_ * nch : NC_ * (nch + 1)],
                        ps[:, NC_ * nch : NC_ * (nch + 1)],
                    )

            def proj_v(mt2, pool, tag):
                """V rows for two token tiles (fills both PSUM banks of the
                ring slot - a half-written ring tile loses the WAR edge on the
                untouched bank)."""
                ps = _ps_tile(pool, tag)
                for i, mt in enumerate(mt2):
                    for kc in range(CT):
                        nc.tensor.matmul(
                            ps[:, NC_ * i : NC_ * (i + 1)],
                            xpe_bf[kc][:, 128 * mt : 128 * (mt + 1)],
                            wv_t[kc][:],
                            start=(kc == 0),
                            stop=(kc == CT - 1),
                        )
                for i, mt in enumerate(mt2):
                    hv = vt_bf[mt][:].rearrange("p (h e) -> p h e", e=D + 1)
                    nc.vector.tensor_copy(
                        hv[:, :, 0:D],
                        ps[:, NC_ * i : NC_ * (i + 1)].rearrange("p (h e) -> p h e", e=D),
                    )

            # Filler projection units, scheduled into (pair, mt) slots.
            # Pair 0 has no PV interleave, so its fillers use the pv pool;
            # later pairs' fillers borrow st-pool buffers (the pv buffers
            # are held by the in-flight PV accumulation -> would deadlock).
            fill_at = {
                (0, 0): lambda: proj_qk(wq_t, q_bf, 1, st_ps, "st"),
                (0, 1): lambda: proj_qk(wk_t, k_bf, 1, st_ps, "st"),
                (0, 2): lambda: proj_v((0, 1), st_ps, "st"),
                (0, 3): lambda: proj_v((2, 3), st_ps, "st"),
                (0, 4): lambda: proj_v((4, 5), st_ps, "st"),
                (0, 5): lambda: proj_v((6, 7), st_ps, "st"),
                (1, 1): lambda: proj_qk(wq_t, q_bf, 2, st_ps, "st"),
                (1, 5): lambda: proj_qk(wk_t, k_bf, 2, st_ps, "st"),
                (2, 1): lambda: proj_qk(wq_t, q_bf, 3, st_ps, "st"),
                (2, 3): lambda: proj_qk(wk_t, k_bf, 3, st_ps, "st"),
            }

            # attention state
            attn_bf = [
                attn_pool.tile([128, N], BF16, tag=f"attnbf{ct}", name=f"attnbf{ct}")
                for ct in range(CT)
            ]
            attn2 = [
                attn_pool.tile([128, N], BF16, tag=f"attn2_{ct}", name=f"attn2_{ct}")
                for ct in range(CT)
            ]
            recip_dram = dram_pool.tile([NH, N], BF16, tag="recipd")
            den2_of = [
                norm_pool.tile([2, N], F32, tag=f"den2_{p % 2}", name=f"den2_{p}")
                for p in range(NP)
            ]
            e_of = {}   # (pair, half) -> list of e tiles per mt
            pv_of = {}  # (pair, half) -> PV psum accumulator

            def scores_pair(p, mt):
                sts = []
                for half in range(2):
                    st = st_ps.tile(
                        [128, N], F32, tag="st", name=f"st{p}_{half}_{mt}"
                    )
                    sts.append(st)
                for nch in range(2):
                    for half in range(2):
                        lo = D * half
                        nc.tensor.matmul(
                            sts[half][:, NC_ * nch : NC_ * (nch + 1)],
                            k_bf[p][lo : lo + D, 128 * mt : 128 * (mt + 1)],
                            q_bf[p][lo : lo + D, NC_ * nch : NC_ * (nch + 1)],
                            start=True,
                            stop=True,
                        )
                for half in range(2):
                    e_t = e_pool.tile([128, N], BF16, tag="e")
                    nc.scalar.activation(
                        e_t[:], sts[half][:], mybir.ActivationFunctionType.Exp,
                        scale=SCALE,
                    )
                    e_of[(p, half)].append(e_t)

            def pv_slot(p, s):
                # slots 0-3: nch0 over mt pairs; slots 4-7: nch1
                nch = s // 4
                if s % 4 == 0:
                    for half in range(2):
                        pv_of[(p, half)] = pv_ps.tile(
                            [D + 1, NC_], F32,
                            tag=f"pv{half}", name=f"pv{p}_{half}_{nch}",
                        )
                for m in (2 * (s % 4), 2 * (s % 4) + 1):
                    for half in range(2):
                        h = 2 * p + half
                        nc.tensor.matmul(
                            pv_of[(p, half)][:],
                            vt_bf[m][:, (D + 1) * h : (D + 1) * (h + 1)],
                            e_of[(p, half)][m][:, NC_ * nch : NC_ * (nch + 1)],
                            start=(m == 0),
                            stop=(m == MT - 1),
                        )
                if s % 4 == 3:
                    drain_nch(p, nch, last=(p == NP - 1))

            pending_norm = []
            pending_norm3 = []

            def flush_norms():
                # deferred so the in-order vector queue never waits on the
                # broadcast DMA round-trip (bc is long since landed by now)
                for ct, bc in pending_norm:
                    nc.vector.tensor_tensor(
                        out=attn2[ct][:], in0=attn_bf[ct][:], in1=bc[:],
                        op=mybir.AluOpType.mult,
                    )
                pending_norm.clear()

            def drain_nch(p, nch, last=False):
                ct = p
                sl = slice(NC_ * nch, NC_ * (nch + 1))
                den2 = den2_of[p]
                for half in range(2):
                    pv = pv_of[(p, half)]
                    den_sb = norm_pool.tile(
                        [128, NC_], F32, tag=f"den{half}", name=f"den{p}_{half}_{nch}"
                    )
                    nc.vector.tensor_copy(den_sb[D : D + 1, :], pv[D : D + 1, :])
                    dq = nc.scalar if last else nc.sync
                    dq.dma_start(den2[half : half + 1, sl], den_sb[D : D + 1, :])
                    if half == 0:
                        nc.vector.tensor_copy(attn_bf[ct][0:D, sl], pv[0:D, :])
                    else:
                        tb = tb_pool.tile([D, NC_], BF16, tag="tb")
                        nc.vector.tensor_copy(tb[:], pv[0:D, :])
                        dq.dma_start(attn_bf[ct][D : 2 * D, sl], tb[:])
                if last:
                    # finish this query-chunk's normalize. nch0 broadcasts via
                    # DRAM (latency hides under the nch1 PV pass, and the pv
                    # bank frees immediately so that pass can start); nch1
                    # broadcasts on the PE. nch0's multiply defers past the
                    # nch1 chain setup so the in-order vector queue never
                    # blocks on the DRAM round-trip.
                    rpf_n = norm_pool.tile(
                        [2, NC_], F32, tag=f"rpfn{nch}", name=f"rpfn{nch}"
                    )
                    nc.vector.reciprocal_approx_fast(rpf_n[:], den2[:, sl])
                    rp_n = norm_pool.tile(
                        [2, NC_], BF16, tag=f"rpn{nch}", name=f"rpn{nch}"
                    )
                    nc.vector.tensor_copy(rp_n[:], rpf_n[:])
                    if nch == 0:
                        bc = attn_pool.tile(
                            [128, NC_], BF16, tag="bc3n0", name="bc3n0"
                        )
                        nc.scalar.dma_start(recip_dram[6:8, 0:NC_], rp_n[:])
                        for half in range(2):
                            nc.scalar.dma_start(
                                bc[D * half : D * (half + 1), :],
                                recip_dram[6 + half : 7 + half, 0:NC_].to_broadcast(
                                    (D, NC_)
                                ),
                            )
                        pending_norm3.append((sl, bc))
                    else:
                        bc_n = pv_ps.tile(
                            [128, NC_], F32, tag="pv1", name="bcn1"
                        )
                        nc.tensor.matmul(
                            bc_n[:], em_sb[:], rp_n[:], start=True, stop=True
                        )
                        for psl, pbc in pending_norm3:
                            nc.vector.tensor_tensor(
                                out=attn2[ct][:, psl], in0=attn_bf[ct][:, psl],
                                in1=pbc[:], op=mybir.AluOpType.mult,
                            )
                        pending_norm3.clear()
                        nc.vector.tensor_tensor(
                            out=attn2[ct][:, sl], in0=attn_bf[ct][:, sl], in1=bc_n[:],
                            op=mybir.AluOpType.mult,
                        )

            def drain_final(p, last=False):
                ct = p
                flush_norms()
                den2 = den2_of[p]
                rpf = norm_pool.tile([2, N], F32, tag=f"rpf{p % 2}", name=f"rpf{p}")
                nc.vector.reciprocal_approx_fast(rpf[:], den2[:])
                rp = norm_pool.tile([2, N], BF16, tag=f"rp{p % 2}", name=f"rp{p}")
                nc.vector.tensor_copy(rp[:], rpf[:])
                bc = attn_pool.tile(
                    [128, N], BF16, tag=f"bc{p % 2}", name=f"bc{p}"
                )
                nc.sync.dma_start(recip_dram[2 * p : 2 * p + 2, :], rp[:])
                for half in range(2):
                    nc.sync.dma_start(
                        bc[D * half : D * (half + 1), :],
                        recip_dram[2 * p + half : 2 * p + half + 1, :].to_broadcast(
                            (D, N)
                        ),
                    )
                pending_norm.append((ct, bc))

            def out_kc(ps, ct, kc, nchs, start, stop):
                for nch in nchs:
                    nc.tensor.matmul(
                        ps[:, NC_ * nch : NC_ * (nch + 1)],
                        wo_t[kc][:, 128 * ct : 128 * (ct + 1)],
                        attn2[kc][:, NC_ * nch : NC_ * (nch + 1)],
                        start=start,
                        stop=stop,
                    )

            def out_body(ct):
                # kc 0-2 need only the first three pairs' attn2 -> can pre-run
                # while the last pair's PV/normalize chain completes
                ps = st_ps.tile([128, N], F32, tag="st", name=f"yps{ct}")
                for kc in range(CT - 1):
                    out_kc(ps, ct, kc, (0, 1), kc == 0, False)
                return ps

            def out_finish(ct, ps, nch):
                out_kc(ps, ct, CT - 1, (nch,), False, True)

            def out_evac(ct, ps):
                for nch in range(2):
                    yt = y_pool.tile([128, NC_], BF16, tag="y")
                    nc.vector.tensor_scalar_add(
                        yt[:], ps[:, NC_ * nch : NC_ * (nch + 1)], bo_t[ct][:]
                    )
                    for h in range(2):
                        sl = slice(NC_ * nch + 256 * h, NC_ * nch + 256 * (h + 1))
                        eng = nc.sync if h == 0 else nc.scalar
                        eng.dma_start(
                            y_ext[128 * ct : 128 * (ct + 1), sl],
                            yt[:, 256 * h : 256 * (h + 1)],
                        )

            # ---- lead-in projections ----
            proj_qk(wq_t, q_bf, 0, st_ps, "st")
            proj_qk(wk_t, k_bf, 0, st_ps, "st")

            # ---- pair pipeline ----
            for p in range(NP):
                e_of[(p, 0)] = []
                e_of[(p, 1)] = []
                for mt in range(MT):
                    scores_pair(p, mt)
                    if p > 0:
                        pv_slot(p - 1, mt)
                    fi = fill_at.get((p, mt))
                    if fi is not None:
                        fi()
                if p > 0:
                    drain_final(p - 1, last=False)
                    del e_of[(p - 1, 0)], e_of[(p - 1, 1)]

            # ---- tail: PV of last pair (overlaps its exp stream), drain,
            # then the output projections (all need every pair's attn2) ----
            lp = NP - 1
            flush_norms()
            for s in range(MT):
                pv_slot(lp, s)
            pss = {}
            for ct in range(3):
                pss[ct] = out_body(ct)
            for ct in range(3):
                out_finish(ct, pss[ct], 0)
            out_finish(0, pss[0], 1)
            out_evac(0, pss[0])
            pss[3] = out_body(3)
            out_finish(3, pss[3], 0)
            for ct in range(1, CT):
                out_finish(ct, pss[ct], 1)
                out_evac(ct, pss[ct])

    nc.compile()
    return nc


_PROGRAM = None


def make_in_maps(x, wq, wk, wv, wo, bo):
    bf = ml_dtypes.bfloat16
    pe32 = _pe_table()
    wqT = np.ascontiguousarray(np.asarray(wq).T).astype(bf)
    wkT = np.ascontiguousarray(np.asarray(wk).T).astype(bf)
    wvT = np.ascontiguousarray(np.asarray(wv).T).astype(bf)
    woT = np.ascontiguousarray(np.asarray(wo).T).astype(bf)
    bo2 = np.ascontiguousarray(np.asarray(bo, dtype=np.float32).reshape(C, 1))
    em = np.zeros((2, 128), dtype=np.float32)
    em[0, 0:D] = 1.0
    em[1, D : 2 * D] = 1.0
    x = np.asarray(x, dtype=np.float32)
    return [
        {
            "x": np.ascontiguousarray(x[b].reshape(C, N) + pe32).astype(bf),
            "wqT": wqT,
            "wkT": wkT,
            "wvT": wvT,
            "woT": woT,
            "bo": bo2,
            "ematrix": em.astype(bf),
        }
        for b in range(B)
    ]


def kernel(x, wq, wk, wv, wo, bo):
    global _PROGRAM
    if _PROGRAM is None:
        _PROGRAM = _build_program()
    nc = _PROGRAM

    in_maps = make_in_maps(x, wq, wk, wv, wo, bo)
    res = run_bass_kernel_spmd(nc, in_maps, list(range(B)))
    out = np.stack(
        [np.asarray(res.results[b]["y"]).reshape(C, H, W) for b in range(B)]
    )
    return out.astype(np.float32)

